# revision 1
# baseline (speedup 1.0000x reference)
"""GRU4Rec forward on 8 Trainium2 cores.

Structure (two NEFF launches):
  1. scan:  data-parallel over batch (128 rows/core). Embedding gather via
     indirect DMA, all GRU gate matmuls in fp32 on PE in a transposed
     ([hidden, batch]) layout so the recurrence needs no per-step transposes.
     Outputs the final hidden state hT [128, 128] per core.
  2. logits: vocab-parallel (12800 padded rows/core). Tied-embedding matmul
     in fp32r (1 cycle/row on PE), staged in SBUF, streamed out with large
     DMAs. Host concatenates the vocab shards.
"""
import os

import numpy as np

import concourse.bass as bass
import concourse.bacc as bacc
import concourse.mybir as mybir
import concourse.tile as tile
from concourse.bass_utils import run_bass_kernel_spmd
from concourse.masks import make_identity

NCORES = 8
B, T, H = 1024, 50, 128
V1 = 100001
BL = B // NCORES          # batch rows per core in the scan
VSTEP = 12500             # vocab shard stride
VS = 12800                # padded vocab shard (25 tiles of 512)
VT = VS // 512

f32 = mybir.dt.float32
f32r = mybir.dt.float32r
i32 = mybir.dt.int32
AF = mybir.ActivationFunctionType
ALU = mybir.AluOpType

_cache = {}
LAST_EXEC_NS = []


def _build_scan(full_mask: bool):
    nc = bacc.Bacc(None, target_bir_lowering=False)
    ids_d = nc.dram_tensor("ids", [BL, T], i32, kind="ExternalInput")
    emb_d = nc.dram_tensor("emb", [V1, H], f32, kind="ExternalInput")
    wx_d = nc.dram_tensor("wx", [H, 3 * H], f32, kind="ExternalInput")
    wr_d = nc.dram_tensor("wr", [H, 3 * H], f32, kind="ExternalInput")
    bias_d = nc.dram_tensor("bias", [H, 5], f32, kind="ExternalInput")
    if not full_mask:
        mask_d = nc.dram_tensor("maskT", [T, BL], f32, kind="ExternalInput")
    ht_d = nc.dram_tensor("ht", [H, BL], f32, kind="ExternalOutput")

    with tile.TileContext(nc) as tc:
        with (
            tc.tile_pool(name="const", bufs=1) as cp,
            tc.tile_pool(name="xg", bufs=6) as xgp,
            tc.tile_pool(name="xt", bufs=1) as xtp,
            tc.tile_pool(name="gate", bufs=3) as gp,
            tc.tile_pool(name="state", bufs=2) as sp,
            tc.tile_pool(name="pst", bufs=2, space="PSUM") as pst,
            tc.tile_pool(name="psg", bufs=2, space="PSUM") as psg,
        ):
            ident = cp.tile([128, 128], f32)
            make_identity(nc, ident[:])
            ids_s = cp.tile([BL, T], i32)
            nc.sync.dma_start(ids_s[:], ids_d[:])
            wx_s = cp.tile([H, 3 * H], f32)
            nc.sync.dma_start(wx_s[:], wx_d[:])
            wr_s = cp.tile([H, 3 * H], f32)
            nc.sync.dma_start(wr_s[:], wr_d[:])
            bias_s = cp.tile([H, 5], f32)
            nc.sync.dma_start(bias_s[:], bias_d[:])
            if not full_mask:
                mask_s = cp.tile([T, BL], f32)
                nc.sync.dma_start(mask_s[:], mask_d[:])

            xT = xtp.tile([H, T * BL], f32)

            hT = sp.tile([H, BL], f32, tag="h")
            nc.gpsimd.memset(hT[:], 0.0)

            def phase_a(t):
                xg = xgp.tile([BL, H], f32, tag="xg")
                nc.gpsimd.indirect_dma_start(
                    out=xg[:],
                    out_offset=None,
                    in_=emb_d[:],
                    in_offset=bass.IndirectOffsetOnAxis(
                        ap=ids_s[:, t : t + 1], axis=0
                    ),
                )
                pt = pst.tile([128, 128], f32, tag="pt")
                nc.tensor.transpose(pt[:], xg[:], ident[:])
                nc.vector.tensor_copy(out=xT[:, t * BL : (t + 1) * BL], in_=pt[:])

            LOOKAHEAD = 4
            for t in range(min(LOOKAHEAD, T)):
                phase_a(t)

            for t in range(T):
                if t + LOOKAHEAD < T:
                    phase_a(t + LOOKAHEAD)
                xTt = xT[:, t * BL : (t + 1) * BL]
                # psum regions: zx = [z | xh] in one bank; r, rh in their own
                ps_zx = psg.tile([128, 256], f32, tag="pszx")
                ps_r = psg.tile([128, 128], f32, tag="psr")
                ps_h = psg.tile([128, 128], f32, tag="psh")
                # input projections (independent of state; can run early)
                nc.tensor.matmul(ps_zx[:, 0:128], wx_s[:, 0:128], xTt,
                                 start=True, stop=False)
                nc.tensor.matmul(ps_zx[:, 128:256], wx_s[:, 256:384], xTt,
                                 start=True, stop=True)
                nc.tensor.matmul(ps_r[:], wx_s[:, 128:256], xTt,
                                 start=True, stop=False)
                # recurrent projections (depend on hT from previous step)
                nc.tensor.matmul(ps_r[:], wr_s[:, 128:256], hT[:],
                                 start=False, stop=True)
                nc.tensor.matmul(ps_zx[:, 0:128], wr_s[:, 0:128], hT[:],
                                 start=False, stop=True)
                nc.tensor.matmul(ps_h[:], wr_s[:, 256:384], hT[:],
                                 start=True, stop=True)

                r = gp.tile([128, BL], f32, tag="r")
                nc.scalar.activation(r[:], ps_r[:], AF.Sigmoid,
                                     bias=bias_s[:, 2:3])
                z = gp.tile([128, BL], f32, tag="z")
                nc.scalar.activation(z[:], ps_zx[:, 0:128], AF.Sigmoid,
                                     bias=bias_s[:, 0:1])
                w = gp.tile([128, BL], f32, tag="w")
                nc.scalar.activation(w[:], ps_zx[:, 0:128], AF.Sigmoid,
                                     bias=bias_s[:, 1:2], scale=-1.0)
                t1 = gp.tile([128, BL], f32, tag="t1")
                nc.vector.scalar_tensor_tensor(
                    out=t1[:], in0=ps_h[:], scalar=bias_s[:, 3:4], in1=r[:],
                    op0=ALU.add, op1=ALU.mult)
                t2 = gp.tile([128, BL], f32, tag="t2")
                nc.vector.tensor_tensor(out=t2[:], in0=t1[:],
                                        in1=ps_zx[:, 128:256], op=ALU.add)
                hh = gp.tile([128, BL], f32, tag="hh")
                nc.scalar.activation(hh[:], t2[:], AF.Tanh,
                                     bias=bias_s[:, 4:5])
                a = gp.tile([128, BL], f32, tag="a")
                nc.vector.tensor_tensor(out=a[:], in0=z[:], in1=hT[:],
                                        op=ALU.mult)
                u = gp.tile([128, BL], f32, tag="u")
                nc.vector.tensor_tensor(out=u[:], in0=w[:], in1=hh[:],
                                        op=ALU.mult)
                h_new = sp.tile([H, BL], f32, tag="h")
                nc.vector.tensor_tensor(out=h_new[:], in0=a[:], in1=u[:],
                                        op=ALU.add)
                if not full_mask:
                    mb = mask_s[t : t + 1, :].to_broadcast([128, BL])
                    hm = gp.tile([128, BL], f32, tag="hm")
                    nc.vector.tensor_tensor(out=hm[:], in0=h_new[:], in1=mb[:],
                                            op=ALU.mult)
                    km = gp.tile([128, BL], f32, tag="km")
                    nc.vector.scalar_tensor_tensor(
                        out=km[:], in0=mb[:], scalar=-1.0, in1=hT[:],
                        op0=ALU.add, op1=ALU.mult)
                    h_masked = sp.tile([H, BL], f32, tag="h")
                    nc.vector.tensor_tensor(out=h_masked[:], in0=hm[:],
                                            in1=km[:], op=ALU.subtract)
                    hT = h_masked
                else:
                    hT = h_new

            nc.sync.dma_start(ht_d[:], hT[:])

    nc.finalize()
    return nc


def _build_logits():
    nc = bacc.Bacc(None, target_bir_lowering=False)
    ht_d = nc.dram_tensor("ht", [H, B], f32r, kind="ExternalInput")
    ev_d = nc.dram_tensor("embv", [VS, H], f32, kind="ExternalInput")
    out_d = nc.dram_tensor("out", [B, VS], f32, kind="ExternalOutput")

    with tile.TileContext(nc) as tc:
        with (
            tc.tile_pool(name="const", bufs=1) as cp,
            tc.tile_pool(name="ld", bufs=6) as ldp,
            tc.tile_pool(name="big", bufs=1) as bigp,
            tc.tile_pool(name="stg", bufs=2) as stgp,
            tc.tile_pool(name="pst", bufs=4, space="PSUM") as pst,
            tc.tile_pool(name="pso", bufs=4, space="PSUM") as pso,
        ):
            ident = cp.tile([128, 128], f32)
            make_identity(nc, ident[:])
            ht_s = cp.tile([H, B], f32r)
            nc.sync.dma_start(ht_s[:], ht_d[:])

            embT = bigp.tile([H, VS], f32r)
            for c in range(VS // 128):
                eg = ldp.tile([128, H], f32, tag="eg")
                nc.sync.dma_start(eg[:], ev_d[c * 128 : (c + 1) * 128, :])
                pt = pst.tile([128, 128], f32, tag="pt")
                nc.tensor.transpose(pt[:], eg[:], ident[:])
                nc.vector.tensor_copy(out=embT[:, c * 128 : (c + 1) * 128],
                                      in_=pt[:])

            for b in range(B // 128):
                stg = stgp.tile([128, VS], f32, tag="stg")
                lhsT = ht_s[:, b * 128 : (b + 1) * 128]
                for v in range(VT):
                    po = pso.tile([128, 512], f32, tag="po")
                    nc.tensor.matmul(po[:], lhsT,
                                     embT[:, v * 512 : (v + 1) * 512],
                                     start=True, stop=True)
                    nc.vector.tensor_copy(out=stg[:, v * 512 : (v + 1) * 512],
                                          in_=po[:])
                nc.sync.dma_start(out_d[b * 128 : (b + 1) * 128, :], stg[:])

    nc.finalize()
    return nc


def _run(nc, in_maps, trace):
    res = run_bass_kernel_spmd(nc, in_maps, core_ids=list(range(NCORES)),
                               trace=trace)
    if trace:
        LAST_EXEC_NS.append(res.exec_time_ns)
    return res


def kernel(**inputs):
    ids = np.asarray(inputs["input_ids"])
    maskv = np.asarray(inputs["input_mask"])
    emb = np.ascontiguousarray(np.asarray(inputs["emb_table"], dtype=np.float32))
    wx = np.ascontiguousarray(np.asarray(inputs["gru_kernel"], dtype=np.float32))
    wr = np.ascontiguousarray(np.asarray(inputs["gru_rec_kernel"], dtype=np.float32))
    gb = np.asarray(inputs["gru_bias"], dtype=np.float32)
    assert ids.shape == (B, T) and emb.shape == (V1, H)

    trace = bool(os.environ.get("GRU_TRACE"))
    LAST_EXEC_NS.clear()

    lengths = (np.asarray(maskv) != 0).astype(np.int64).sum(axis=1)
    full_mask = bool((lengths == T).all())

    bi, br_ = gb[0], gb[1]
    bias_np = np.stack(
        [
            bi[0:128] + br_[0:128],
            -(bi[0:128] + br_[0:128]),
            bi[128:256] + br_[128:256],
            br_[256:384],
            bi[256:384],
        ],
        axis=1,
    ).astype(np.float32)

    key = ("scan", full_mask)
    if key not in _cache:
        _cache[key] = _build_scan(full_mask)
    nc1 = _cache[key]

    ids32 = ids.astype(np.int32)
    in_maps1 = []
    for c in range(NCORES):
        sl = slice(c * BL, (c + 1) * BL)
        m = {
            "ids": np.ascontiguousarray(ids32[sl]),
            "emb": emb,
            "wx": wx,
            "wr": wr,
            "bias": bias_np,
        }
        if not full_mask:
            m["maskT"] = np.ascontiguousarray(
                maskv[sl].T.astype(np.float32))
        in_maps1.append(m)
    res1 = _run(nc1, in_maps1, trace)
    hT_full = np.ascontiguousarray(
        np.concatenate([res1.results[c]["ht"] for c in range(NCORES)], axis=1))

    if "logits" not in _cache:
        _cache["logits"] = _build_logits()
    nc2 = _cache["logits"]

    in_maps2 = []
    for c in range(NCORES):
        lo = c * VSTEP
        hi = min(lo + VS, V1)
        shard = np.zeros((VS, H), np.float32)
        shard[: hi - lo] = emb[lo:hi]
        in_maps2.append({"ht": hT_full, "embv": shard})
    res2 = _run(nc2, in_maps2, trace)

    logits = np.empty((B, V1), dtype=np.float32)
    for c in range(NCORES):
        lo = c * VSTEP
        hi = min(lo + VS, V1)
        logits[:, lo:hi] = res2.results[c]["out"][:, : hi - lo]
    return logits


# revision 2
# speedup vs baseline: 1.0291x; 1.0291x over previous
"""GRU4Rec forward on 8 Trainium2 cores.

Structure (two NEFF launches):
  1. scan:  data-parallel over batch (128 rows/core). Embedding gather via
     indirect DMA, all GRU gate matmuls in fp32 on PE in a transposed
     ([hidden, batch]) layout so the recurrence needs no per-step transposes.
     Outputs the final hidden state hT [128, 128] per core.
  2. logits: vocab-parallel (12800 padded rows/core). Tied-embedding matmul
     in fp32r (1 cycle/row on PE), staged in SBUF, streamed out with large
     DMAs. Host concatenates the vocab shards.
"""
import os

import numpy as np

import concourse.bass as bass
import concourse.bacc as bacc
import concourse.mybir as mybir
import concourse.tile as tile
from concourse.bass_utils import run_bass_kernel_spmd
from concourse.masks import make_identity

NCORES = 8
B, T, H = 1024, 50, 128
V1 = 100001
BL = B // NCORES          # batch rows per core in the scan
VSTEP = 12500             # vocab shard stride
VS = 12800                # padded vocab shard (25 tiles of 512)
VT = VS // 512

f32 = mybir.dt.float32
f32r = mybir.dt.float32r
i32 = mybir.dt.int32
AF = mybir.ActivationFunctionType
ALU = mybir.AluOpType

_cache = {}
LAST_EXEC_NS = []


def _build_scan(full_mask: bool):
    nc = bacc.Bacc(None, target_bir_lowering=False)
    ids_d = nc.dram_tensor("ids", [BL, T], i32, kind="ExternalInput")
    emb_d = nc.dram_tensor("emb", [V1, H], f32, kind="ExternalInput")
    wx_d = nc.dram_tensor("wx", [H, 3 * H], f32, kind="ExternalInput")
    wr_d = nc.dram_tensor("wr", [H, 3 * H], f32, kind="ExternalInput")
    bias_d = nc.dram_tensor("bias", [H, 5], f32, kind="ExternalInput")
    if not full_mask:
        mask_d = nc.dram_tensor("maskT", [T, BL], f32, kind="ExternalInput")
    ht_d = nc.dram_tensor("ht", [H, BL], f32, kind="ExternalOutput")

    with tile.TileContext(nc) as tc:
        with (
            tc.tile_pool(name="const", bufs=1) as cp,
            tc.tile_pool(name="xg", bufs=6) as xgp,
            tc.tile_pool(name="xt", bufs=1) as xtp,
            tc.tile_pool(name="gate", bufs=3) as gp,
            tc.tile_pool(name="state", bufs=2) as sp,
            tc.tile_pool(name="pst", bufs=1, space="PSUM") as pst,
            tc.tile_pool(name="psg", bufs=2, space="PSUM") as psg,
        ):
            ident = cp.tile([128, 128], f32)
            make_identity(nc, ident[:])
            ids_s = cp.tile([BL, T], i32)
            nc.sync.dma_start(ids_s[:], ids_d[:])
            wx_s = cp.tile([H, 3 * H], f32)
            nc.sync.dma_start(wx_s[:], wx_d[:])
            wr_s = cp.tile([H, 3 * H], f32)
            nc.sync.dma_start(wr_s[:], wr_d[:])
            bias_s = cp.tile([H, 5], f32)
            nc.sync.dma_start(bias_s[:], bias_d[:])
            if not full_mask:
                mask_s = cp.tile([T, BL], f32)
                nc.sync.dma_start(mask_s[:], mask_d[:])

            xT = xtp.tile([H, T * BL], f32)

            hT = sp.tile([H, BL], f32, tag="h")
            nc.gpsimd.memset(hT[:], 0.0)

            def phase_a(t):
                xg = xgp.tile([BL, H], f32, tag="xg")
                nc.gpsimd.indirect_dma_start(
                    out=xg[:],
                    out_offset=None,
                    in_=emb_d[:],
                    in_offset=bass.IndirectOffsetOnAxis(
                        ap=ids_s[:, t : t + 1], axis=0
                    ),
                )
                pt = pst.tile([128, 128], f32, tag="pt")
                nc.tensor.transpose(pt[:], xg[:], ident[:])
                nc.vector.tensor_copy(out=xT[:, t * BL : (t + 1) * BL], in_=pt[:])

            LOOKAHEAD = 4
            for t in range(min(LOOKAHEAD, T)):
                phase_a(t)

            for t in range(T):
                if t + LOOKAHEAD < T:
                    phase_a(t + LOOKAHEAD)
                xTt = xT[:, t * BL : (t + 1) * BL]
                # One PSUM bank per accumulation group: a start=True matmul
                # resets accumulation state bank-wide (data in other regions
                # survives, but has_written does not), so the z and r groups
                # each get an exclusive bank; the single-write xh/rh regions
                # share one.
                ps_z = psg.tile([128, 128], f32, tag="psz")
                ps_r = psg.tile([128, 128], f32, tag="psr")
                ps_c = psg.tile([128, 256], f32, tag="psc")  # xh | rh
                # input projections (independent of state; can run early)
                nc.tensor.matmul(ps_z[:], wx_s[:, 0:128], xTt,
                                 start=True, stop=False)
                nc.tensor.matmul(ps_c[:, 0:128], wx_s[:, 256:384], xTt,
                                 start=True, stop=True)
                nc.tensor.matmul(ps_r[:], wx_s[:, 128:256], xTt,
                                 start=True, stop=False)
                # recurrent projections (depend on hT from previous step)
                nc.tensor.matmul(ps_r[:], wr_s[:, 128:256], hT[:],
                                 start=False, stop=True)
                nc.tensor.matmul(ps_z[:], wr_s[:, 0:128], hT[:],
                                 start=False, stop=True)
                nc.tensor.matmul(ps_c[:, 128:256], wr_s[:, 256:384], hT[:],
                                 start=True, stop=True)

                r = gp.tile([128, BL], f32, tag="r")
                nc.scalar.activation(r[:], ps_r[:], AF.Sigmoid,
                                     bias=bias_s[:, 2:3])
                z = gp.tile([128, BL], f32, tag="z")
                nc.scalar.activation(z[:], ps_z[:], AF.Sigmoid,
                                     bias=bias_s[:, 0:1])
                w = gp.tile([128, BL], f32, tag="w")
                nc.scalar.activation(w[:], ps_z[:], AF.Sigmoid,
                                     bias=bias_s[:, 1:2], scale=-1.0)
                t1 = gp.tile([128, BL], f32, tag="t1")
                nc.vector.scalar_tensor_tensor(
                    out=t1[:], in0=ps_c[:, 128:256], scalar=bias_s[:, 3:4],
                    in1=r[:], op0=ALU.add, op1=ALU.mult)
                t2 = gp.tile([128, BL], f32, tag="t2")
                nc.vector.tensor_tensor(out=t2[:], in0=t1[:],
                                        in1=ps_c[:, 0:128], op=ALU.add)
                hh = gp.tile([128, BL], f32, tag="hh")
                nc.scalar.activation(hh[:], t2[:], AF.Tanh,
                                     bias=bias_s[:, 4:5])
                a = gp.tile([128, BL], f32, tag="a")
                nc.vector.tensor_tensor(out=a[:], in0=z[:], in1=hT[:],
                                        op=ALU.mult)
                u = gp.tile([128, BL], f32, tag="u")
                nc.vector.tensor_tensor(out=u[:], in0=w[:], in1=hh[:],
                                        op=ALU.mult)
                h_new = sp.tile([H, BL], f32, tag="h")
                nc.vector.tensor_tensor(out=h_new[:], in0=a[:], in1=u[:],
                                        op=ALU.add)
                if not full_mask:
                    mb = mask_s[t : t + 1, :].to_broadcast([128, BL])
                    hm = gp.tile([128, BL], f32, tag="hm")
                    nc.vector.tensor_tensor(out=hm[:], in0=h_new[:], in1=mb[:],
                                            op=ALU.mult)
                    km = gp.tile([128, BL], f32, tag="km")
                    nc.vector.scalar_tensor_tensor(
                        out=km[:], in0=mb[:], scalar=-1.0, in1=hT[:],
                        op0=ALU.add, op1=ALU.mult)
                    h_masked = sp.tile([H, BL], f32, tag="h")
                    nc.vector.tensor_tensor(out=h_masked[:], in0=hm[:],
                                            in1=km[:], op=ALU.subtract)
                    hT = h_masked
                else:
                    hT = h_new

            nc.sync.dma_start(ht_d[:], hT[:])

    nc.finalize()
    return nc


def _build_logits():
    nc = bacc.Bacc(None, target_bir_lowering=False)
    ht_d = nc.dram_tensor("ht", [H, B], f32r, kind="ExternalInput")
    ev_d = nc.dram_tensor("embv", [VS, H], f32, kind="ExternalInput")
    out_d = nc.dram_tensor("out", [B, VS], f32, kind="ExternalOutput")

    with tile.TileContext(nc) as tc:
        with (
            tc.tile_pool(name="const", bufs=1) as cp,
            tc.tile_pool(name="ld", bufs=6) as ldp,
            tc.tile_pool(name="big", bufs=1) as bigp,
            tc.tile_pool(name="stg", bufs=2) as stgp,
            tc.tile_pool(name="pst", bufs=4, space="PSUM") as pst,
            tc.tile_pool(name="pso", bufs=4, space="PSUM") as pso,
        ):
            ident = cp.tile([128, 128], f32)
            make_identity(nc, ident[:])
            ht_s = cp.tile([H, B], f32r)
            nc.sync.dma_start(ht_s[:], ht_d[:])

            embT = bigp.tile([H, VS], f32r)
            for c in range(VS // 128):
                eg = ldp.tile([128, H], f32, tag="eg")
                nc.sync.dma_start(eg[:], ev_d[c * 128 : (c + 1) * 128, :])
                pt = pst.tile([128, 128], f32, tag="pt")
                nc.tensor.transpose(pt[:], eg[:], ident[:])
                nc.vector.tensor_copy(out=embT[:, c * 128 : (c + 1) * 128],
                                      in_=pt[:])

            for b in range(B // 128):
                stg = stgp.tile([128, VS], f32, tag="stg")
                lhsT = ht_s[:, b * 128 : (b + 1) * 128]
                for v in range(VT):
                    po = pso.tile([128, 512], f32, tag="po")
                    nc.tensor.matmul(po[:], lhsT,
                                     embT[:, v * 512 : (v + 1) * 512],
                                     start=True, stop=True)
                    nc.vector.tensor_copy(out=stg[:, v * 512 : (v + 1) * 512],
                                          in_=po[:])
                nc.sync.dma_start(out_d[b * 128 : (b + 1) * 128, :], stg[:])

    nc.finalize()
    return nc


def _run(nc, in_maps, trace):
    res = run_bass_kernel_spmd(nc, in_maps, core_ids=list(range(NCORES)),
                               trace=trace)
    if trace:
        LAST_EXEC_NS.append(res.exec_time_ns)
    return res


def kernel(**inputs):
    ids = np.asarray(inputs["input_ids"])
    maskv = np.asarray(inputs["input_mask"])
    emb = np.ascontiguousarray(np.asarray(inputs["emb_table"], dtype=np.float32))
    wx = np.ascontiguousarray(np.asarray(inputs["gru_kernel"], dtype=np.float32))
    wr = np.ascontiguousarray(np.asarray(inputs["gru_rec_kernel"], dtype=np.float32))
    gb = np.asarray(inputs["gru_bias"], dtype=np.float32)
    assert ids.shape == (B, T) and emb.shape == (V1, H)

    trace = bool(os.environ.get("GRU_TRACE"))
    LAST_EXEC_NS.clear()

    lengths = (np.asarray(maskv) != 0).astype(np.int64).sum(axis=1)
    full_mask = bool((lengths == T).all())

    bi, br_ = gb[0], gb[1]
    bias_np = np.stack(
        [
            bi[0:128] + br_[0:128],
            -(bi[0:128] + br_[0:128]),
            bi[128:256] + br_[128:256],
            br_[256:384],
            bi[256:384],
        ],
        axis=1,
    ).astype(np.float32)

    key = ("scan", full_mask)
    if key not in _cache:
        _cache[key] = _build_scan(full_mask)
    nc1 = _cache[key]

    ids32 = ids.astype(np.int32)
    in_maps1 = []
    for c in range(NCORES):
        sl = slice(c * BL, (c + 1) * BL)
        m = {
            "ids": np.ascontiguousarray(ids32[sl]),
            "emb": emb,
            "wx": wx,
            "wr": wr,
            "bias": bias_np,
        }
        if not full_mask:
            m["maskT"] = np.ascontiguousarray(
                maskv[sl].T.astype(np.float32))
        in_maps1.append(m)
    res1 = _run(nc1, in_maps1, trace)
    hT_full = np.ascontiguousarray(
        np.concatenate([res1.results[c]["ht"] for c in range(NCORES)], axis=1))

    if "logits" not in _cache:
        _cache["logits"] = _build_logits()
    nc2 = _cache["logits"]

    in_maps2 = []
    for c in range(NCORES):
        lo = c * VSTEP
        hi = min(lo + VS, V1)
        shard = np.zeros((VS, H), np.float32)
        shard[: hi - lo] = emb[lo:hi]
        in_maps2.append({"ht": hT_full, "embv": shard})
    res2 = _run(nc2, in_maps2, trace)

    logits = np.empty((B, V1), dtype=np.float32)
    for c in range(NCORES):
        lo = c * VSTEP
        hi = min(lo + VS, V1)
        logits[:, lo:hi] = res2.results[c]["out"][:, : hi - lo]
    return logits
